# revision 10
# baseline (speedup 1.0000x reference)
"""ASTRF kernel for Trainium2 (8 NeuronCores, axon) — fp8 DoubleRow, 12 DR/half.

Math: out[b,o,t] = sum_{i,w} weight[o,i,w] * xs[b,i,t-w] + bias[o]
where xs[b,i,src[b,s]] = x[b,i,s] (scatter of events to onsets).

Banded block-matmul decomposition: t = 64m + q, q = 8c + z', contraction
K = (u'=8, i=16) = 128, N = (z', o) = 512 per half c, 9 window-products per
half.  Operands split hi/lo into fp8(e4m3) pairs; per half we emit 12
MatmulPerfMode.DoubleRow matmuls:
  - 4 main-pair DRs: (Xh*Wh[t], Xh*Wh[t+1])
  - 7 correction DRs (windows t=1..7 only): (Xh*Wl[t], Xl*Wh[t])
  - 1 leftover DR: odd c -> (Xh*Wh[8], Xl*Wh[8]) via dup window 9;
                   even c -> (Xl*Wl[t*], Xh*Wh[t*])
The hi*lo corrections for edge windows t=0 and t=8 are dropped: they carry
~12% of the tap variance, leaving absmax rel err ~1.84e-2 < 2e-2 and saving
2 DRs per half (cost model: DR costs out_free_size * 0.5 * pe_cycle).

SBUF layouts (plane-major x so the hi plane ships as one early DMA; chunk
dim stored reversed, a_hat = 7-a, so banded products ascend in memory):
  x_sb  [128 p=(u_lo,i), j, v3=(l,h,l), a_hat, col]  fp8
  wsh_sb[128 p=(u',i),  win10, v2=(l,h), 512]        fp8  (win9 = dup win8)

Outputs drain as bf16; host applies the 2^-16 split prescale and the bias.
"""

import sys

for _p in ("/opt/trn_rl_repo", "/root/.axon_site/_ro/trn_rl_repo"):
    if _p not in sys.path:
        sys.path.insert(0, _p)

import numpy as np

B, I, S = 4, 16, 4096
O, W = 64, 64
T = 32768
NBLK = T // 64            # 512 blocks per batch
N_CORES = 8
XS, WS = 16.0, 4096.0     # pow2 prescales for fp8 quantization
XCOL = 144                # x col dim padded so DR slot strides are 16-aligned
OUT_SCALE = 1.0 / (XS * WS)
N_WARM = 6
GA = [1, 2, 3, 4, 5, 0]   # phase-A halves (j=0), supply-ordered waves

_prog_cache = {}


def _mats_for(c):
    """Banded product list for half c: (a, k, col0), t-th entry has window
    k = t.  seg1 = t in [0,c] (col0=1, current block), seg2 = t in [c+1,8]
    (col0=0, previous block).  With a_hat = 7-a, a_hat ascends within each
    segment."""
    mats = [(c, 0, 1)]
    mats += [(c - k, k, 1) for k in range(1, c + 1)]
    mats += [(c + 8 - k, k, 0) for k in range(c + 1, 9)]
    return mats


def _half_plan(c):
    """Return (main_pairs, leftover, c1c2) for half c.

    main_pairs: list of (t, ahat, col0) meaning DR over entries (t, t+1)
    leftover:   (t, ahat, col0): odd c -> t=8 (XhWh[8],XlWh[8]);
                even c -> (XlWl[t*],XhWh[t*])
    c1c2:       list of (t, ahat, col0) correction DRs, windows 1..7 only
    """
    mats = _mats_for(c)
    ahat = [7 - a for (a, k, col) in mats]
    col = [cc for (a, k, cc) in mats]
    seg1 = list(range(0, c + 1))
    seg2 = list(range(c + 1, 9))
    pairs, leftover = [], None
    for seg in (seg1, seg2):
        for idx in range(0, len(seg) - 1, 2):
            t = seg[idx]
            pairs.append((t, ahat[t], col[t]))
        if len(seg) % 2:
            t = seg[-1]
            leftover = (t, ahat[t], col[t])
    assert len(pairs) == 4 and leftover is not None
    c1c2 = [(t, ahat[t], col[t]) for t in range(1, 8)]
    return pairs, leftover, c1c2


def _build_program():
    if "nc" in _prog_cache:
        return _prog_cache["nc"]
    import concourse.bacc as bacc
    import concourse.mybir as mybir
    import concourse.tile as tile

    f8 = mybir.dt.float8e4
    f32 = mybir.dt.float32
    bf16 = mybir.dt.bfloat16
    DR = mybir.MatmulPerfMode.DoubleRow
    nc = bacc.Bacc("TRN2", target_bir_lowering=False, debug=False, num_devices=N_CORES)

    xin = nc.dram_tensor("xin", [128, 2, 3, 8, XCOL], f8, kind="ExternalInput")
    wshd = nc.dram_tensor("wshd", [128, 10, 2, 512], f8, kind="ExternalInput")
    out = nc.dram_tensor("out", [2, 8, 128, 512], bf16, kind="ExternalOutput")

    plans = {c: _half_plan(c) for c in range(8)}

    with tile.TileContext(nc) as tc:
        with (
            tc.tile_pool(name="const", bufs=1) as cpool,
            tc.tile_pool(name="stage", bufs=4) as spool,
            tc.tile_pool(name="psum", bufs=7, space="PSUM") as ppool,
            tc.tile_pool(name="psumw", bufs=1, space="PSUM") as wpool,
        ):
            x_sb = cpool.tile([128, 2, 3, 8, XCOL], f8, tag="x")
            wsh_sb = cpool.tile([128, 10, 2, 512], f8, tag="wsh")
            scr = cpool.tile([128, 128], f32, tag="scr")
            wps = wpool.tile([128, 512], f32, tag="wps", name="warm_ps")
            nc.vector.memset(scr[:], 0.0)
            for _w in range(N_WARM):
                nc.tensor.matmul(
                    out=wps[:, 0:128],
                    lhsT=scr[:],
                    rhs=scr[:],
                    start=(_w == 0),
                    stop=(_w == N_WARM - 1),
                )

            # --- input DMAs, supply-ordered (SP issues; HWDGE serializes
            # at 625ns per DMA, transfers at ~360B/ns aggregate, +900ns sem).
            # Phase-A waves consume in exactly this order. ---
            nc.sync.dma_start(out=x_sb[:, 0, 1], in_=xin[:, 0, 1])  # x hi j0
            nc.sync.dma_start(out=wsh_sb[:, 0:3, 1, :], in_=wshd[:, 0:3, 1, :])
            nc.sync.dma_start(out=wsh_sb[:, 3:6, 1, :], in_=wshd[:, 3:6, 1, :])
            nc.sync.dma_start(out=wsh_sb[:, 6:10, 1, :], in_=wshd[:, 6:10, 1, :])
            nc.sync.dma_start(out=wsh_sb[:, 0:5, 0, :], in_=wshd[:, 0:5, 0, :])
            nc.sync.dma_start(out=x_sb[:, 0, 2], in_=xin[:, 0, 2])  # x lo j0
            nc.sync.dma_start(out=wsh_sb[:, 5:10, 0, :], in_=wshd[:, 5:10, 0, :])
            nc.sync.dma_start(out=x_sb[:, 0, 0], in_=xin[:, 0, 0])  # x lo0 j0
            nc.sync.dma_start(out=x_sb[:, 1], in_=xin[:, 1])        # x j1

            def dr(j, ps, kind, t, ahat, col0, start, stop, nlo=0, nhi=512):
                if kind == "mp":    # (Xh*Wh[t], Xh*Wh[t+1])
                    lhsT = x_sb[:, j, 1, ahat : ahat + 2, col0 : col0 + 128]
                    rhs = wsh_sb[:, t : t + 2, 1, nlo:nhi]
                elif kind == "cc":  # (Xh*Wl[t], Xl*Wh[t])
                    lhsT = x_sb[:, j, 1:3, ahat, col0 : col0 + 128]
                    rhs = wsh_sb[:, t, 0:2, nlo:nhi]
                elif kind == "lo":  # odd-c leftover: (Xh*Wh[8], Xl*Wh[8])
                    lhsT = x_sb[:, j, 1:3, ahat, col0 : col0 + 128]
                    rhs = wsh_sb[:, 8:10, 1, nlo:nhi]
                else:               # "le" even-c leftover: (Xl*Wl, Xh*Wh)
                    lhsT = x_sb[:, j, 0:2, ahat, col0 : col0 + 128]
                    rhs = wsh_sb[:, t, 0:2, nlo:nhi]
                nc.tensor.matmul(
                    out=ps[:, nlo:nhi],
                    lhsT=lhsT,
                    rhs=rhs,
                    start=start,
                    stop=stop,
                    perf_mode=DR,
                )

            def drain(j, c, ps, eng, nlo=0, nhi=512):
                stage = spool.tile(
                    [128, nhi - nlo], bf16, tag="stage", name=f"st{j}_{c}_{nlo}"
                )
                if eng == 0:
                    nc.vector.tensor_copy(out=stage[:], in_=ps[:, nlo:nhi])
                else:
                    nc.scalar.copy(out=stage[:], in_=ps[:, nlo:nhi])
                nc.sync.dma_start(out=out[j, c, :, nlo:nhi], in_=stage[:])

            # ---------------- phase A: 6 interleaved j=0 halves ----------
            ps_of = {}
            for c in GA:
                ps_of[c] = ppool.tile([128, 512], f32, tag="ps", name=f"psA{c}")
            for w_idx in range(4):
                wave = GA if w_idx != 2 else [1, 3, 5, 2, 4, 0]
                for c in wave:
                    t, ahat, col0 = plans[c][0][w_idx]
                    dr(0, ps_of[c], "mp", t, ahat, col0, w_idx == 0, False)
            for c in GA:
                if c % 2 == 1:  # odd-c leftovers need only hi planes
                    t, ahat, col0 = plans[c][1]
                    dr(0, ps_of[c], "lo", t, ahat, col0, False, False)
            for t_idx in range(7):
                for c in GA:
                    t, ahat, col0 = plans[c][2][t_idx]
                    last = (t_idx == 6) and (c % 2 == 1)
                    dr(0, ps_of[c], "cc", t, ahat, col0, False, last)
            for n, c in enumerate(c for c in GA if c % 2 == 1):
                drain(0, c, ps_of[c], n % 2)
            for n, c in enumerate(c for c in GA if c % 2 == 0):
                t, ahat, col0 = plans[c][1]
                dr(0, ps_of[c], "le", t, ahat, col0, False, True)
                drain(0, c, ps_of[c], n % 2)

            # ---------------- phase B: remaining halves, sequential ------
            def emit_half(j, c, ps, nlo=0, nhi=512):
                pairs, leftover, c1c2 = plans[c]
                for w_idx, (t, ahat, col0) in enumerate(pairs):
                    dr(j, ps, "mp", t, ahat, col0, w_idx == 0, False, nlo, nhi)
                if c % 2 == 1:
                    t, ahat, col0 = leftover
                    dr(j, ps, "lo", t, ahat, col0, False, False, nlo, nhi)
                for k, (t, ahat, col0) in enumerate(c1c2):
                    last = (k == 6) and (c % 2 == 1)
                    dr(j, ps, "cc", t, ahat, col0, False, last, nlo, nhi)
                if c % 2 == 0:
                    t, ahat, col0 = leftover
                    dr(j, ps, "le", t, ahat, col0, False, True, nlo, nhi)

            rest = [(0, 6), (0, 7)] + [(1, c) for c in range(8)]
            for n, (j, c) in enumerate(rest):
                if (j, c) == rest[-1]:
                    # final half in two column strips so the drain chain of
                    # strip A overlaps strip B's compute (shorter tail)
                    psn = ppool.tile([128, 512], f32, tag="ps", name="psNarrow")
                    emit_half(j, c, psn, 0, 256)
                    drain(j, c, psn, 0, 0, 256)
                    emit_half(j, c, psn, 256, 512)
                    drain(j, c, psn, 1, 256, 512)
                    continue
                ps = ppool.tile([128, 512], f32, tag="ps", name=f"psB{j}_{c}")
                emit_half(j, c, ps)
                drain(j, c, ps, n % 2)

    nc.compile()
    _prog_cache["nc"] = nc
    return nc


def _quant_split(a, scale):
    """Return (hi, lo) fp8(e4m3) split of a*scale, as float8_e4m3 arrays."""
    import ml_dtypes

    f8 = ml_dtypes.float8_e4m3
    hi = (a * scale).astype(f8)
    lo = (a * scale - hi.astype(np.float32)).astype(f8)
    return hi, lo


def _host_pack(x, weight, sourceIdx):
    """Build per-core device inputs from full inputs."""
    import ml_dtypes

    f8 = ml_dtypes.float8_e4m3
    xh, xl = _quant_split(np.asarray(x, np.float32), XS)

    # scatter into blocked plane-major layout
    # xs6[b, u_lo, i, v3=(l,h,l), a_hat, col=m+1]
    xs6 = np.zeros((B, 8, I, 3, 8, NBLK + 1), f8)
    src = np.asarray(sourceIdx, np.int64)
    for b in range(B):
        t = src[b]
        m = (t >> 6).astype(np.int64)
        u = (t & 63).astype(np.int64)
        ahat = 7 - (u >> 3)
        ulo = u & 7
        for i in range(I):
            xs6[b, ulo, i, 0, ahat, m + 1] = xl[b, i]
            xs6[b, ulo, i, 1, ahat, m + 1] = xh[b, i]
            xs6[b, ulo, i, 2, ahat, m + 1] = xl[b, i]

    x_cores = []
    for core in range(N_CORES):
        b, h = divmod(core, 2)
        tmp = xs6[b].reshape(128, 3, 8, NBLK + 1)
        arr = np.zeros((128, 2, 3, 8, XCOL), f8)
        for j in range(2):
            g = 2 * h + j
            arr[:, j, :, :, :129] = tmp[:, :, :, 128 * g : 128 * g + 129]
        x_cores.append(np.ascontiguousarray(arr))

    # shifted weights: wsh[p=(u',i), win, v2=(l,h), (zz',o)]
    wgt = np.asarray(weight, np.float32)  # (O, I, W)
    wh, wl = _quant_split(wgt, WS)
    zz = np.arange(72)
    up = np.arange(8)
    idx = zz[None, :] - up[:, None]              # (8 u', 72 zz')
    valid = (idx >= 0) & (idx < W)
    planes = []
    for wv in (wl, wh):
        g = wv.astype(np.float32)[:, :, np.clip(idx, 0, W - 1)] * valid[None, None]
        planes.append(g.transpose(2, 1, 3, 0).reshape(128, 9, 512))
    wsh_host = np.empty((128, 10, 2, 512), f8)
    wsh_host[:, :9, 0, :] = planes[0].astype(f8)
    wsh_host[:, :9, 1, :] = planes[1].astype(f8)
    wsh_host[:, 9] = wsh_host[:, 8]  # dup win 8 for the odd-c leftover DR
    return x_cores, np.ascontiguousarray(wsh_host)


def kernel(x, weight, bias, sourceIdx, nRealLen, _trace=False, _trace_out=None):
    import jax

    from concourse import bass_utils

    if len(jax.devices()) < N_CORES:
        jax.config.update("jax_platforms", "axon")
        try:
            import jax.extend.backend

            jax.extend.backend.clear_backends()
        except Exception:
            pass
        assert len(jax.devices()) >= N_CORES, (
            f"need {N_CORES} neuron cores, have {jax.devices()}"
        )

    nRealLen = int(nRealLen)
    assert nRealLen == T, f"kernel hardcoded for nRealLen={T}, got {nRealLen}"
    x_cores, wsh_host = _host_pack(x, weight, sourceIdx)
    nc = _build_program()
    in_maps = [{"xin": x_cores[c], "wshd": wsh_host} for c in range(N_CORES)]
    res = bass_utils.run_bass_kernel_spmd(
        nc,
        in_maps,
        core_ids=list(range(N_CORES)),
        trace=_trace,
        trace_cores=list(range(N_CORES)) if _trace else None,
    )
    if _trace_out is not None:
        _trace_out.append(res)
    bias_f = np.asarray(bias, np.float32)
    out_full = np.empty((B, O, T), np.float32)
    for core in range(N_CORES):
        b, h = divmod(core, 2)
        r = np.asarray(res.results[core]["out"], dtype=np.float32)  # (2,8,128,512)
        r6 = r.reshape(2, 8, 128, 8, 64)  # [j, c, m, q', o]
        for j in range(2):
            g = 2 * h + j
            # t' = m*64 + c*8 + q'
            seg = r6[j].transpose(3, 1, 0, 2).reshape(64, 8192)
            out_full[b, :, g * 8192 : (g + 1) * 8192] = seg
    out_full *= OUT_SCALE
    out_full += bias_f[None, :, None]
    return out_full


# revision 12
# speedup vs baseline: 1.0184x; 1.0184x over previous
"""ASTRF kernel for Trainium2 (8 NeuronCores, axon) — fp8 DoubleRow, 12 DR/half.

Math: out[b,o,t] = sum_{i,w} weight[o,i,w] * xs[b,i,t-w] + bias[o]
where xs[b,i,src[b,s]] = x[b,i,s] (scatter of events to onsets).

Banded block-matmul decomposition: t = 64m + q, q = 8c + z', contraction
K = (u'=8, i=16) = 128, N = (z', o) = 512 per half c, 9 window-products per
half.  Operands split hi/lo into fp8(e4m3) pairs; per half we emit 12
MatmulPerfMode.DoubleRow matmuls:
  - 4 main-pair DRs: (Xh*Wh[t], Xh*Wh[t+1])
  - 7 correction DRs (windows t=1..7 only): (Xh*Wl[t], Xl*Wh[t])
  - 1 leftover DR: odd c -> (Xh*Wh[8], Xl*Wh[8]) via dup window 9;
                   even c -> (Xl*Wl[t*], Xh*Wh[t*])
The hi*lo corrections for edge windows t=0 and t=8 are dropped: they carry
~12% of the tap variance, leaving absmax rel err ~1.84e-2 < 2e-2 and saving
2 DRs per half (cost model: DR costs out_free_size * 0.5 * pe_cycle).

SBUF layouts (plane-major x so the hi plane ships as one early DMA; chunk
dim stored reversed, a_hat = 7-a, so banded products ascend in memory):
  x_sb  [128 p=(u_lo,i), j, v3=(l,h,l), a_hat, col]  fp8
  wsh_sb[128 p=(u',i),  win10, v2=(l,h), 512]        fp8  (win9 = dup win8)

Outputs drain as bf16; host applies the 2^-16 split prescale and the bias.
"""

import sys

for _p in ("/opt/trn_rl_repo", "/root/.axon_site/_ro/trn_rl_repo"):
    if _p not in sys.path:
        sys.path.insert(0, _p)

import numpy as np

B, I, S = 4, 16, 4096
O, W = 64, 64
T = 32768
NBLK = T // 64            # 512 blocks per batch
N_CORES = 8
XS, WS = 16.0, 4096.0     # pow2 prescales for fp8 quantization
XCOL = 144                # x col dim padded so DR slot strides are 16-aligned
OUT_SCALE = 1.0 / (XS * WS)
N_WARM = 7
GA = [1, 2, 3, 4, 5, 0]   # phase-A halves (j=0), supply-ordered waves

_prog_cache = {}


def _mats_for(c):
    """Banded product list for half c: (a, k, col0), t-th entry has window
    k = t.  seg1 = t in [0,c] (col0=1, current block), seg2 = t in [c+1,8]
    (col0=0, previous block).  With a_hat = 7-a, a_hat ascends within each
    segment."""
    mats = [(c, 0, 1)]
    mats += [(c - k, k, 1) for k in range(1, c + 1)]
    mats += [(c + 8 - k, k, 0) for k in range(c + 1, 9)]
    return mats


def _half_plan(c):
    """Return (main_pairs, leftover, c1c2) for half c.

    main_pairs: list of (t, ahat, col0) meaning DR over entries (t, t+1)
    leftover:   (t, ahat, col0): odd c -> t=8 (XhWh[8],XlWh[8]);
                even c -> (XlWl[t*],XhWh[t*])
    c1c2:       list of (t, ahat, col0) correction DRs, windows 1..7 only
    """
    mats = _mats_for(c)
    ahat = [7 - a for (a, k, col) in mats]
    col = [cc for (a, k, cc) in mats]
    seg1 = list(range(0, c + 1))
    seg2 = list(range(c + 1, 9))
    pairs, leftover = [], None
    for seg in (seg1, seg2):
        for idx in range(0, len(seg) - 1, 2):
            t = seg[idx]
            pairs.append((t, ahat[t], col[t]))
        if len(seg) % 2:
            t = seg[-1]
            leftover = (t, ahat[t], col[t])
    assert len(pairs) == 4 and leftover is not None
    c1c2 = [(t, ahat[t], col[t]) for t in range(1, 8)]
    return pairs, leftover, c1c2


def _build_program():
    if "nc" in _prog_cache:
        return _prog_cache["nc"]
    import concourse.bacc as bacc
    import concourse.mybir as mybir
    import concourse.tile as tile

    f8 = mybir.dt.float8e4
    f32 = mybir.dt.float32
    bf16 = mybir.dt.bfloat16
    DR = mybir.MatmulPerfMode.DoubleRow
    nc = bacc.Bacc("TRN2", target_bir_lowering=False, debug=False, num_devices=N_CORES)

    xin = nc.dram_tensor("xin", [128, 2, 3, 8, XCOL], f8, kind="ExternalInput")
    wshd = nc.dram_tensor("wshd", [128, 10, 2, 512], f8, kind="ExternalInput")
    out = nc.dram_tensor("out", [2, 8, 128, 512], bf16, kind="ExternalOutput")

    plans = {c: _half_plan(c) for c in range(8)}

    with tile.TileContext(nc) as tc:
        with (
            tc.tile_pool(name="const", bufs=1) as cpool,
            tc.tile_pool(name="stage", bufs=4) as spool,
            tc.tile_pool(name="psum", bufs=7, space="PSUM") as ppool,
            tc.tile_pool(name="psumw", bufs=1, space="PSUM") as wpool,
        ):
            x_sb = cpool.tile([128, 2, 3, 8, XCOL], f8, tag="x")
            wsh_sb = cpool.tile([128, 10, 2, 512], f8, tag="wsh")
            scr = cpool.tile([128, 128], f32, tag="scr")
            wps = wpool.tile([128, 512], f32, tag="wps", name="warm_ps")
            nc.vector.memset(scr[:], 0.0)
            for _w in range(N_WARM):
                nc.tensor.matmul(
                    out=wps[:, 0:128],
                    lhsT=scr[:],
                    rhs=scr[:],
                    start=(_w == 0),
                    stop=(_w == N_WARM - 1),
                )

            # --- input DMAs, supply-ordered (SP issues; HWDGE serializes
            # at 625ns per DMA, transfers at ~360B/ns aggregate, +900ns sem).
            # Phase-A waves consume in exactly this order. ---
            nc.sync.dma_start(out=x_sb[:, 0, 1], in_=xin[:, 0, 1])  # x hi j0
            nc.sync.dma_start(out=wsh_sb[:, 0:3, 1, :], in_=wshd[:, 0:3, 1, :])
            nc.sync.dma_start(out=wsh_sb[:, 3:6, 1, :], in_=wshd[:, 3:6, 1, :])
            nc.sync.dma_start(out=wsh_sb[:, 6:10, 1, :], in_=wshd[:, 6:10, 1, :])
            nc.sync.dma_start(out=wsh_sb[:, 0:5, 0, :], in_=wshd[:, 0:5, 0, :])
            nc.sync.dma_start(out=x_sb[:, 0, 2], in_=xin[:, 0, 2])  # x lo j0
            nc.sync.dma_start(out=wsh_sb[:, 5:10, 0, :], in_=wshd[:, 5:10, 0, :])
            nc.sync.dma_start(out=x_sb[:, 0, 0], in_=xin[:, 0, 0])  # x lo0 j0
            nc.sync.dma_start(out=x_sb[:, 1], in_=xin[:, 1])        # x j1

            def dr(j, ps, kind, t, ahat, col0, start, stop, nlo=0, nhi=512):
                if kind == "mp":    # (Xh*Wh[t], Xh*Wh[t+1])
                    lhsT = x_sb[:, j, 1, ahat : ahat + 2, col0 : col0 + 128]
                    rhs = wsh_sb[:, t : t + 2, 1, nlo:nhi]
                elif kind == "cc":  # (Xh*Wl[t], Xl*Wh[t])
                    lhsT = x_sb[:, j, 1:3, ahat, col0 : col0 + 128]
                    rhs = wsh_sb[:, t, 0:2, nlo:nhi]
                elif kind == "lo":  # odd-c leftover: (Xh*Wh[8], Xl*Wh[8])
                    lhsT = x_sb[:, j, 1:3, ahat, col0 : col0 + 128]
                    rhs = wsh_sb[:, 8:10, 1, nlo:nhi]
                else:               # "le" even-c leftover: (Xl*Wl, Xh*Wh)
                    lhsT = x_sb[:, j, 0:2, ahat, col0 : col0 + 128]
                    rhs = wsh_sb[:, t, 0:2, nlo:nhi]
                nc.tensor.matmul(
                    out=ps[:, nlo:nhi],
                    lhsT=lhsT,
                    rhs=rhs,
                    start=start,
                    stop=stop,
                    perf_mode=DR,
                )

            def drain(j, c, ps, eng, nlo=0, nhi=512):
                stage = spool.tile(
                    [128, nhi - nlo], bf16, tag="stage", name=f"st{j}_{c}_{nlo}"
                )
                if eng == 0:
                    nc.vector.tensor_copy(out=stage[:], in_=ps[:, nlo:nhi])
                else:
                    nc.scalar.copy(out=stage[:], in_=ps[:, nlo:nhi])
                nc.sync.dma_start(out=out[j, c, :, nlo:nhi], in_=stage[:])

            # ---------------- phase A: 6 interleaved j=0 halves ----------
            ps_of = {}
            for c in GA:
                ps_of[c] = ppool.tile([128, 512], f32, tag="ps", name=f"psA{c}")
            for w_idx in range(4):
                wave = GA if w_idx != 2 else [1, 3, 5, 2, 4, 0]
                for c in wave:
                    t, ahat, col0 = plans[c][0][w_idx]
                    dr(0, ps_of[c], "mp", t, ahat, col0, w_idx == 0, False)
            for c in GA:
                if c % 2 == 1:  # odd-c leftovers need only hi planes
                    t, ahat, col0 = plans[c][1]
                    dr(0, ps_of[c], "lo", t, ahat, col0, False, False)
            for t_idx in range(7):
                for c in GA:
                    t, ahat, col0 = plans[c][2][t_idx]
                    last = (t_idx == 6) and (c % 2 == 1)
                    dr(0, ps_of[c], "cc", t, ahat, col0, False, last)
            for n, c in enumerate(c for c in GA if c % 2 == 1):
                drain(0, c, ps_of[c], n % 2)
            for n, c in enumerate(c for c in GA if c % 2 == 0):
                t, ahat, col0 = plans[c][1]
                dr(0, ps_of[c], "le", t, ahat, col0, False, True)
                drain(0, c, ps_of[c], n % 2)

            # ---------------- phase B: remaining halves, sequential ------
            def emit_half(j, c, ps, nlo=0, nhi=512):
                pairs, leftover, c1c2 = plans[c]
                for w_idx, (t, ahat, col0) in enumerate(pairs):
                    dr(j, ps, "mp", t, ahat, col0, w_idx == 0, False, nlo, nhi)
                if c % 2 == 1:
                    t, ahat, col0 = leftover
                    dr(j, ps, "lo", t, ahat, col0, False, False, nlo, nhi)
                for k, (t, ahat, col0) in enumerate(c1c2):
                    last = (k == 6) and (c % 2 == 1)
                    dr(j, ps, "cc", t, ahat, col0, False, last, nlo, nhi)
                if c % 2 == 0:
                    t, ahat, col0 = leftover
                    dr(j, ps, "le", t, ahat, col0, False, True, nlo, nhi)

            rest = [(0, 6), (0, 7)] + [(1, c) for c in range(8)]
            for n, (j, c) in enumerate(rest):
                if (j, c) == rest[-1]:
                    # final half in two column strips with separate psum
                    # tiles so strip B's matmuls don't wait on strip A's
                    # drain copy (shorter tail)
                    psA = ppool.tile([128, 512], f32, tag="ps", name="psNarrowA")
                    psB = ppool.tile([128, 512], f32, tag="ps", name="psNarrowB")
                    emit_half(j, c, psA, 0, 256)
                    drain(j, c, psA, 0, 0, 256)
                    emit_half(j, c, psB, 256, 512)
                    drain(j, c, psB, 1, 256, 512)
                    continue
                ps = ppool.tile([128, 512], f32, tag="ps", name=f"psB{j}_{c}")
                emit_half(j, c, ps)
                drain(j, c, ps, n % 2)

    nc.compile()
    _prog_cache["nc"] = nc
    return nc


def _quant_split(a, scale):
    """Return (hi, lo) fp8(e4m3) split of a*scale, as float8_e4m3 arrays."""
    import ml_dtypes

    f8 = ml_dtypes.float8_e4m3
    hi = (a * scale).astype(f8)
    lo = (a * scale - hi.astype(np.float32)).astype(f8)
    return hi, lo


def _host_pack(x, weight, sourceIdx):
    """Build per-core device inputs from full inputs."""
    import ml_dtypes

    f8 = ml_dtypes.float8_e4m3
    xh, xl = _quant_split(np.asarray(x, np.float32), XS)

    # scatter into blocked plane-major layout
    # xs6[b, u_lo, i, v3=(l,h,l), a_hat, col=m+1]
    xs6 = np.zeros((B, 8, I, 3, 8, NBLK + 1), f8)
    src = np.asarray(sourceIdx, np.int64)
    for b in range(B):
        t = src[b]
        m = (t >> 6).astype(np.int64)
        u = (t & 63).astype(np.int64)
        ahat = 7 - (u >> 3)
        ulo = u & 7
        for i in range(I):
            xs6[b, ulo, i, 0, ahat, m + 1] = xl[b, i]
            xs6[b, ulo, i, 1, ahat, m + 1] = xh[b, i]
            xs6[b, ulo, i, 2, ahat, m + 1] = xl[b, i]

    x_cores = []
    for core in range(N_CORES):
        b, h = divmod(core, 2)
        tmp = xs6[b].reshape(128, 3, 8, NBLK + 1)
        arr = np.zeros((128, 2, 3, 8, XCOL), f8)
        for j in range(2):
            g = 2 * h + j
            arr[:, j, :, :, :129] = tmp[:, :, :, 128 * g : 128 * g + 129]
        x_cores.append(np.ascontiguousarray(arr))

    # shifted weights: wsh[p=(u',i), win, v2=(l,h), (zz',o)]
    wgt = np.asarray(weight, np.float32)  # (O, I, W)
    wh, wl = _quant_split(wgt, WS)
    zz = np.arange(72)
    up = np.arange(8)
    idx = zz[None, :] - up[:, None]              # (8 u', 72 zz')
    valid = (idx >= 0) & (idx < W)
    planes = []
    for wv in (wl, wh):
        g = wv.astype(np.float32)[:, :, np.clip(idx, 0, W - 1)] * valid[None, None]
        planes.append(g.transpose(2, 1, 3, 0).reshape(128, 9, 512))
    wsh_host = np.empty((128, 10, 2, 512), f8)
    wsh_host[:, :9, 0, :] = planes[0].astype(f8)
    wsh_host[:, :9, 1, :] = planes[1].astype(f8)
    wsh_host[:, 9] = wsh_host[:, 8]  # dup win 8 for the odd-c leftover DR
    return x_cores, np.ascontiguousarray(wsh_host)


def kernel(x, weight, bias, sourceIdx, nRealLen, _trace=False, _trace_out=None):
    import jax

    from concourse import bass_utils

    if len(jax.devices()) < N_CORES:
        jax.config.update("jax_platforms", "axon")
        try:
            import jax.extend.backend

            jax.extend.backend.clear_backends()
        except Exception:
            pass
        assert len(jax.devices()) >= N_CORES, (
            f"need {N_CORES} neuron cores, have {jax.devices()}"
        )

    nRealLen = int(nRealLen)
    assert nRealLen == T, f"kernel hardcoded for nRealLen={T}, got {nRealLen}"
    x_cores, wsh_host = _host_pack(x, weight, sourceIdx)
    nc = _build_program()
    in_maps = [{"xin": x_cores[c], "wshd": wsh_host} for c in range(N_CORES)]
    res = bass_utils.run_bass_kernel_spmd(
        nc,
        in_maps,
        core_ids=list(range(N_CORES)),
        trace=_trace,
        trace_cores=list(range(N_CORES)) if _trace else None,
    )
    if _trace_out is not None:
        _trace_out.append(res)
    bias_f = np.asarray(bias, np.float32)
    out_full = np.empty((B, O, T), np.float32)
    for core in range(N_CORES):
        b, h = divmod(core, 2)
        r = np.asarray(res.results[core]["out"], dtype=np.float32)  # (2,8,128,512)
        r6 = r.reshape(2, 8, 128, 8, 64)  # [j, c, m, q', o]
        for j in range(2):
            g = 2 * h + j
            # t' = m*64 + c*8 + q'
            seg = r6[j].transpose(3, 1, 0, 2).reshape(64, 8192)
            out_full[b, :, g * 8192 : (g + 1) * 8192] = seg
    out_full *= OUT_SCALE
    out_full += bias_f[None, :, None]
    return out_full
